# revision 25
# baseline (speedup 1.0000x reference)
"""DiffusionGraphConv Trainium2 kernel (fp16 matmul pipeline).

Math (per batch b, support s, A = supports[s]):
  x0 = concat(inputs, state)                      # [N, F=128]
  reference out = sum_k x_k @ W_k  (+bias), k in {x0, x1_s0, x2_s0, x1_s1, x2_s1}
  with x1 = A x0, x2 = 2 A A x0 - x0, W_k = weight[f*5+k, :].

Restructured to avoid any on-chip transposes:
  out = x0 @ What + bias + sum_s A_s @ (x0 @ W1_s + A_s @ (x0 @ (2*W2_s)))
  with What = W_0 - W_2 - W_4, (W1_s, W2_s) = (W_1, W_2) for s=0, (W_3, W_4) for s=1.

All matmul operands are fp16 (PSUM accumulation stays fp32): fp16 streams
1 col/cycle like f32r, but its 128x128 stationary load goes through FWL and
hides under the previous matmul's streaming -- ~216 ns per 512-wide matmul
instead of f32r's ~273 ns (f32r self-loads its 4-byte weights serially).
fp16 also halves input DMA bytes, so both supports load upfront.
End-to-end quantization error ~5e-4 (fp32 accumulate, fp16 operands).

Layouts (per core, batch-sharded B_local = 8):
  x0T  DRAM [b=8, F=128, m=1024]   (host-staged transpose; lhsT tiles for x0@W)
  atT  DRAM [s=2, m=1024, n=1024]  (host-staged A^T; lhsT tiles for A-mults)
  All A-mult operands keep the node index on partitions -> layout-consistent
  chain, final out written per node-chunk as [n, b, o] blocks.
"""

import sys as _sys
import types as _types

try:
    import antenv.axon_hooks  # noqa: F401
except Exception:
    try:
        import antenv as _antenv

        _m = _types.ModuleType("antenv.axon_hooks")
        _m._hook = None
        _m.set_axon_ntff_profile_hook = lambda h: setattr(_m, "_hook", h)
        _m.get_axon_ntff_profile_hook = lambda: _m._hook
        _sys.modules["antenv.axon_hooks"] = _m
        _antenv.axon_hooks = _m
    except Exception:
        pass

import numpy as np

import concourse.mybir as mybir
import concourse.tile as tile
from concourse import bacc
from concourse.bass_utils import run_bass_kernel_spmd

NCORES = 8
B = 64
BL = B // NCORES  # 8 batches per core
N = 1024
F = 128
O = 128
NCH = N // 128  # 8 node chunks

F16 = mybir.dt.float16
F32 = mybir.dt.float32

WARMUP = 12

_CACHE = {}


def _build():
    if "nc" in _CACHE:
        return _CACHE["nc"]

    nc = bacc.Bacc(trn_type="TRN2", num_devices=NCORES, debug=False)

    x0t_d = nc.dram_tensor("x0t", [BL, F, N], F16, kind="ExternalInput")
    at_d = nc.dram_tensor("at", [2, N, N], F16, kind="ExternalInput")
    # host-prepped: [:,0]=What=W0-W2-W4, [:,1]=W1, [:,2]=2*W2, [:,3]=W3, [:,4]=2*W4
    w_d = nc.dram_tensor("w", [F, 5, O], F16, kind="ExternalInput")
    b_d = nc.dram_tensor("b", [1, BL * O], F32, kind="ExternalInput")  # tiled bias
    out_d = nc.dram_tensor("out", [N, BL, O], F32, kind="ExternalOutput")

    with tile.TileContext(nc) as tc:
        with (
            tc.tile_pool(name="big", bufs=1) as big,
            tc.tile_pool(name="small", bufs=1) as small,
            tc.tile_pool(name="ps_pool", bufs=8, space="PSUM") as ps_pool,
        ):
            # ---- persistent tiles ----
            # wc[:, k, :] = W_k; after prep: k=0 slot -> What, k=2/4 -> 2*W2/2*W4
            wc = small.tile([F, 5, O], F16)
            bias_t = small.tile([1, BL * O], F32)
            b1024 = small.tile([128, BL * O], F32)
            x0t_t = big.tile([F, BL, N], F16)  # 16KB/part
            at_t0 = big.tile([128, NCH, N], F16)  # 16KB/part
            at_t1 = big.tile([128, NCH, N], F16)  # 16KB/part
            st0 = big.tile([128, NCH, BL, 256], F16)  # 32KB/part
            st1 = big.tile([128, NCH, BL, 256], F16)  # 32KB/part
            v0 = big.tile([128, NCH, N], F16)  # 16KB/part
            v1 = big.tile([128, NCH, N], F16)  # 16KB/part
            fins = [
                big.tile([128, N], F32, name=f"fin{ni}") for ni in range(NCH)
            ]  # 32KB/part total

            # ---- PE warm-up: dummy matmuls during the DMA head so HAM
            # un-throttles (1.2 -> 2.4 GHz) before real work starts
            dummy = small.tile([128, 256], F16)
            dsink = small.tile([128, 1], F32)
            nc.vector.memset(dummy[:], 0.0)
            for _ in range(WARMUP):
                pw = ps_pool.tile([128, 256], F32, name="ps_w", tag="ps")
                nc.tensor.matmul(
                    pw[:], dummy[:, 0:128], dummy[:], start=True, stop=True
                )
            nc.vector.tensor_copy(dsink[:], pw[:, 0:1])

            # ---- head input DMAs. The DMA engines round-robin descriptors
            # across ALL outstanding transfers, so everything kicked at once
            # completes together near the end of the transfer window. Kick
            # only what the head needs (wc, x0t b0-3, at0); x0t b4-7 and at1
            # are kicked later, interleaved with the schedule (below).
            # each kick costs ~606ns serialized on the sync sequencer, so
            # consolidate; order by need-time so round-robin completion of
            # concurrent transfers roughly matches consumption order
            nc.sync.dma_start(wc[:], w_d[:])
            nc.sync.dma_start(
                x0t_t[:, 0:2, :], x0t_d[0:2].rearrange("b p n -> p b n")
            )
            nc.sync.dma_start(
                at_t0[:, :, :], at_d[0].rearrange("(a p) n -> p a n", p=128)
            )
            nc.sync.dma_start(
                x0t_t[:, 2:4, :], x0t_d[2:4].rearrange("b p n -> p b n")
            )
            nc.sync.dma_start(bias_t[:], b_d[:])

            # bias broadcast: first consumed by fin0's adds (~40us in)
            nc.gpsimd.partition_broadcast(b1024[:], bias_t[:])

            # ---- Sa step (b, mi): one stationary x0T tile, stream a W pair:
            #   s=0: [W1|2*W2], s=1: [W3|2*W4]; pair -> staging (one cast).
            # The x0@What term is folded into fin0's PSUM groups instead, so
            # the head has no bias/fins dependency and PSUM recycles at cast
            # pace. copies alternate DVE/ACT.
            def sa_step(s, st, b, mi):
                cnt = b * NCH + mi
                ps = ps_pool.tile([128, 512], F32, name="ps_sa", tag="ps")
                nc.tensor.matmul(
                    ps[:, :256],
                    x0t_t[:, b, mi * 128 : (mi + 1) * 128],
                    wc[:, 1:3, :] if s == 0 else wc[:, 3:5, :],
                    start=True,
                    stop=True,
                )
                pair = ps[:, 0:256]
                dst = st[:, mi, b, :]
                flip = (cnt % 8) < 5 if s == 1 else cnt % 2 == 0
                if flip:
                    nc.scalar.copy(dst, pair)
                else:
                    nc.vector.tensor_copy(dst, pair)

            # ---- v bank (ni, h): v_s[ni, h] = A_s @ u_s + w1p_s
            def v_bank(at_t, st, v, ni, h):
                pv = ps_pool.tile([128, 512], F32, name="ps_v", tag="ps")
                for mi in range(NCH):
                    nc.tensor.matmul(
                        pv[:],
                        at_t[:, mi, ni * 128 : (ni + 1) * 128],
                        st[:, mi, 4 * h : 4 * h + 4, 128:256],
                        start=(mi == 0),
                        stop=(mi == NCH - 1),
                    )
                nc.vector.tensor_add(
                    v[:, ni, h * 512 : (h + 1) * 512],
                    pv[:],
                    st[:, ni, 4 * h : 4 * h + 4, 0:128],
                )

            # ---- fin bank (ni, h):
            #   s=0: fin[ni, h] = (A_0 @ v_0 + x0 @ What) + bias
            #        (the per-batch What matmuls accumulate into the same
            #        PSUM group; 128-wide, LDWEIGHTS hides under streaming)
            #   s=1: fin[ni, h] += A_1 @ v_1; then DMA out
            def fin_bank(s, at_t, v, ni, h, dma_split=1):
                pf = ps_pool.tile([128, 512], F32, name="ps_f", tag="ps")
                for mi in range(NCH):
                    nc.tensor.matmul(
                        pf[:],
                        at_t[:, mi, ni * 128 : (ni + 1) * 128],
                        v[:, mi, h * 512 : (h + 1) * 512],
                        start=(mi == 0),
                        stop=(s == 1 and mi == NCH - 1),
                    )
                if s == 0:
                    for bb in range(4):
                        nc.tensor.matmul(
                            pf[:, bb * 128 : (bb + 1) * 128],
                            x0t_t[:, 4 * h + bb, ni * 128 : (ni + 1) * 128],
                            wc[:, 0, :],
                            start=False,
                            stop=(bb == 3),
                        )
                fslc = fins[ni][:, h * 512 : (h + 1) * 512]
                w_ = 512 // dma_split
                for p in range(dma_split):
                    sl = slice(h * 512 + p * w_, h * 512 + (p + 1) * w_)
                    psl = slice(p * w_, (p + 1) * w_)
                    if s == 0:
                        nc.vector.tensor_add(
                            fins[ni][:, sl], pf[:, psl], b1024[:, sl]
                        )
                    else:
                        nc.vector.tensor_add(
                            fins[ni][:, sl], fins[ni][:, sl], pf[:, psl]
                        )
                        # out kicks on the scalar queue: the sync sequencer's
                        # 606ns DIRECT2D kicks would backlog the output drain
                        nc.scalar.dma_start(
                            out_d[
                                ni * 128 : (ni + 1) * 128,
                                4 * h + p * 4 // dma_split : 4 * h
                                + (p + 1) * 4 // dma_split,
                                :,
                            ],
                            fins[ni][:, sl],
                        )

            # ---- schedule (software-pipelined emission) ----
            # S0a half 0 (b 0-3): copy-paced (~212ns/step across DVE+ACT)
            # while x0t/at0 stream in; everything else is PE-bound, so the
            # remaining Sa work interleaves under the A-mult phases.
            for b in range(4):
                for mi in range(NCH):
                    sa_step(0, st0, b, mi)
                if b == 2:
                    # kick x0t b4-5 late enough not to compete with at0
                    nc.sync.dma_start(
                        x0t_t[:, 4:6, :],
                        x0t_d[4:6].rearrange("b p n -> p b n"),
                    )
            # v0 h=0 banks interleaved with S0a half 1 (spreads copies under PE)
            for ni in range(NCH):
                v_bank(at_t0, st0, v0, ni, 0)
                if ni == 0:
                    nc.sync.dma_start(
                        x0t_t[:, 6:8, :],
                        x0t_d[6:8].rearrange("b p n -> p b n"),
                    )
                for mi in range(NCH // 2):
                    sa_step(0, st0, 4 + ni // 2, (ni % 2) * 4 + mi)
            # v0 h=1 banks interleaved with first half of S1a (b 0-3)
            s1_steps = iter([(b, mi) for b in range(BL) for mi in range(NCH)])
            for ni in range(NCH):
                v_bank(at_t0, st0, v0, ni, 1)
                for _ in range(4):
                    b_, mi_ = next(s1_steps)
                    sa_step(1, st1, b_, mi_)

            # fin0 with the rest of S1a packed into its first half (4 per bank)
            for ni in range(NCH):
                for h in range(2):
                    fin_bank(0, at_t0, v0, ni, h)
                    if ni < 4:
                        for _ in range(4):
                            b_, mi_ = next(s1_steps)
                            sa_step(1, st1, b_, mi_)
                if ni == 0:
                    # at1 loads under fin0; one kick
                    nc.sync.dma_start(
                        at_t1[:, :, :],
                        at_d[1].rearrange("(a p) n -> p a n", p=128),
                    )

            for ni in range(NCH):
                v_bank(at_t1, st1, v1, ni, 0)
            for ni in range(NCH):
                v_bank(at_t1, st1, v1, ni, 1)
            for ni in range(NCH):
                for h in range(2):
                    # final bank: drain+DMA in halves to shorten the tail
                    last = ni == NCH - 1 and h == 1
                    fin_bank(1, at_t1, v1, ni, h, dma_split=2 if last else 1)

    nc.compile()
    _CACHE["nc"] = nc
    return nc


def kernel(supports, inputs, state, weight, biases, output_size, _trace=False):
    supports = np.asarray(supports, dtype=np.float32)
    inputs = np.asarray(inputs, dtype=np.float32)
    state = np.asarray(state, dtype=np.float32)
    weight = np.asarray(weight, dtype=np.float32)
    biases = np.asarray(biases, dtype=np.float32)
    O_ = int(output_size)
    assert O_ == O and inputs.shape == (B, N * 64) and supports.shape == (2, N, N)

    nc = _build()

    # host staging (layout + fp16 cast): A^T, x0^T, prepped W, tiled bias row
    at_np = np.ascontiguousarray(supports.transpose(0, 2, 1)).astype(np.float16)
    x0 = np.concatenate(
        [inputs.reshape(B, N, 64), state.reshape(B, N, 64)], axis=2
    )  # [B, N, F]
    x0t = x0.transpose(0, 2, 1)  # [B, F, N] view; per-core slice made contiguous
    wk = weight.reshape(F, 5, O)
    wprep = np.stack(
        [
            wk[:, 0] - wk[:, 2] - wk[:, 4],  # What
            wk[:, 1],
            2.0 * wk[:, 2],
            wk[:, 3],
            2.0 * wk[:, 4],
        ],
        axis=1,
    )
    w16 = np.ascontiguousarray(wprep).astype(np.float16)  # [F, 5, O]
    brow = np.ascontiguousarray(np.tile(biases, BL)[None, :]).astype(np.float32)

    in_maps = []
    for c in range(NCORES):
        in_maps.append(
            {
                "x0t": np.ascontiguousarray(
                    x0t[c * BL : (c + 1) * BL]
                ).astype(np.float16),
                "at": at_np,
                "w": w16,
                "b": brow,
            }
        )

    res = run_bass_kernel_spmd(
        nc, in_maps, core_ids=list(range(NCORES)), trace=_trace
    )
    kernel.last_result = res

    # out per core: [N, BL, O] -> full [B, N*O]
    parts = [res.results[c]["out"] for c in range(NCORES)]
    full = np.concatenate(parts, axis=1)  # [N, B, O]
    return np.ascontiguousarray(full.transpose(1, 0, 2)).reshape(B, N * O_)


# revision 29
# speedup vs baseline: 1.0277x; 1.0277x over previous
"""DiffusionGraphConv Trainium2 kernel (fp16 matmul pipeline).

Math (per batch b, support s, A = supports[s]):
  x0 = concat(inputs, state)                      # [N, F=128]
  reference out = sum_k x_k @ W_k  (+bias), k in {x0, x1_s0, x2_s0, x1_s1, x2_s1}
  with x1 = A x0, x2 = 2 A A x0 - x0, W_k = weight[f*5+k, :].

Restructured to avoid any on-chip transposes:
  out = x0 @ What + bias + sum_s A_s @ (x0 @ W1_s + A_s @ (x0 @ (2*W2_s)))
  with What = W_0 - W_2 - W_4, (W1_s, W2_s) = (W_1, W_2) for s=0, (W_3, W_4) for s=1.

All matmul operands are fp16 (PSUM accumulation stays fp32): fp16 streams
1 col/cycle like f32r, but its 128x128 stationary load goes through FWL and
hides under the previous matmul's streaming -- ~216 ns per 512-wide matmul
instead of f32r's ~273 ns (f32r self-loads its 4-byte weights serially).
fp16 also halves input DMA bytes, so both supports load upfront.
End-to-end quantization error ~5e-4 (fp32 accumulate, fp16 operands).

Layouts (per core, batch-sharded B_local = 8):
  x0T  DRAM [b=8, F=128, m=1024]   (host-staged transpose; lhsT tiles for x0@W)
  atT  DRAM [s=2, m=1024, n=1024]  (host-staged A^T; lhsT tiles for A-mults)
  All A-mult operands keep the node index on partitions -> layout-consistent
  chain, final out written per node-chunk as [n, b, o] blocks.
"""

import sys as _sys
import types as _types

try:
    import antenv.axon_hooks  # noqa: F401
except Exception:
    try:
        import antenv as _antenv

        _m = _types.ModuleType("antenv.axon_hooks")
        _m._hook = None
        _m.set_axon_ntff_profile_hook = lambda h: setattr(_m, "_hook", h)
        _m.get_axon_ntff_profile_hook = lambda: _m._hook
        _sys.modules["antenv.axon_hooks"] = _m
        _antenv.axon_hooks = _m
    except Exception:
        pass

import numpy as np

import concourse.mybir as mybir
import concourse.tile as tile
from concourse import bacc
from concourse.bass_utils import run_bass_kernel_spmd

NCORES = 8
B = 64
BL = B // NCORES  # 8 batches per core
N = 1024
F = 128
O = 128
NCH = N // 128  # 8 node chunks

F16 = mybir.dt.float16
F32 = mybir.dt.float32

WARMUP = 16

_CACHE = {}


def _build():
    if "nc" in _CACHE:
        return _CACHE["nc"]

    nc = bacc.Bacc(trn_type="TRN2", num_devices=NCORES, debug=False)

    x0t_d = nc.dram_tensor("x0t", [BL, F, N], F16, kind="ExternalInput")
    at_d = nc.dram_tensor("at", [2, N, N], F16, kind="ExternalInput")
    # host-prepped: [:,0]=What=W0-W2-W4, [:,1]=W1, [:,2]=2*W2, [:,3]=W3, [:,4]=2*W4
    w_d = nc.dram_tensor("w", [F, 5, O], F16, kind="ExternalInput")
    b_d = nc.dram_tensor("b", [1, BL * O], F32, kind="ExternalInput")  # tiled bias
    out_d = nc.dram_tensor("out", [N, BL, O], F32, kind="ExternalOutput")

    with tile.TileContext(nc) as tc:
        with (
            tc.tile_pool(name="big", bufs=1) as big,
            tc.tile_pool(name="small", bufs=1) as small,
            tc.tile_pool(name="ps_pool", bufs=8, space="PSUM") as ps_pool,
        ):
            # ---- persistent tiles ----
            # wc[:, k, :] = W_k; after prep: k=0 slot -> What, k=2/4 -> 2*W2/2*W4
            wc = small.tile([F, 5, O], F16)
            bias_t = small.tile([1, BL * O], F32)
            b1024 = small.tile([128, BL * O], F32)
            x0t_t = big.tile([F, BL, N], F16)  # 16KB/part
            at_t0 = big.tile([128, NCH, N], F16)  # 16KB/part
            at_t1 = big.tile([128, NCH, N], F16)  # 16KB/part
            st0 = big.tile([128, NCH, BL, 256], F16)  # 32KB/part
            st1 = big.tile([128, NCH, BL, 256], F16)  # 32KB/part
            v0 = big.tile([128, NCH, N], F16)  # 16KB/part
            v1 = big.tile([128, NCH, N], F16)  # 16KB/part
            fins = [
                big.tile([128, N], F32, name=f"fin{ni}") for ni in range(NCH)
            ]  # 32KB/part total

            # ---- PE warm-up: dummy matmuls during the DMA head so HAM
            # un-throttles (1.2 -> 2.4 GHz) before real work starts
            dummy = small.tile([128, 256], F16)
            dsink = small.tile([128, 1], F32)
            nc.vector.memset(dummy[:], 0.0)
            for _ in range(WARMUP):
                pw = ps_pool.tile([128, 256], F32, name="ps_w", tag="ps")
                nc.tensor.matmul(
                    pw[:], dummy[:, 0:128], dummy[:], start=True, stop=True
                )
            nc.vector.tensor_copy(dsink[:], pw[:, 0:1])

            # ---- head input DMAs. The DMA engines round-robin descriptors
            # across ALL outstanding transfers, so everything kicked at once
            # completes together near the end of the transfer window. Kick
            # only what the head needs (wc, x0t b0-3, at0); x0t b4-7 and at1
            # are kicked later, interleaved with the schedule (below).
            # per-transfer kicks (~606ns each on the sync sequencer, cost
            # scales with line count -- consolidated multi-MB kicks stall
            # everything queued behind them, so keep transfers chunked)
            nc.sync.dma_start(wc[:], w_d[:])
            nc.sync.dma_start(bias_t[:], b_d[:])
            for b in range(4):
                nc.sync.dma_start(x0t_t[:, b, :], x0t_d[b])
            for mi in range(NCH):
                nc.sync.dma_start(
                    at_t0[:, mi, :], at_d[0, mi * 128 : (mi + 1) * 128, :]
                )

            # bias broadcast: first consumed by fin0's adds (~40us in)
            nc.gpsimd.partition_broadcast(b1024[:], bias_t[:])

            # ---- Sa step (b, mi): one stationary x0T tile, stream a W pair:
            #   s=0: [W1|2*W2], s=1: [W3|2*W4]; pair -> staging (one cast).
            # The x0@What term is folded into fin0's PSUM groups instead, so
            # the head has no bias/fins dependency and PSUM recycles at cast
            # pace. copies alternate DVE/ACT.
            def sa_step(s, st, b, mi):
                cnt = b * NCH + mi
                ps = ps_pool.tile([128, 512], F32, name="ps_sa", tag="ps")
                nc.tensor.matmul(
                    ps[:, :256],
                    x0t_t[:, b, mi * 128 : (mi + 1) * 128],
                    wc[:, 1:3, :] if s == 0 else wc[:, 3:5, :],
                    start=True,
                    stop=True,
                )
                pair = ps[:, 0:256]
                dst = st[:, mi, b, :]
                flip = (cnt % 8) < 5 if s == 1 else cnt % 2 == 0
                if flip:
                    nc.scalar.copy(dst, pair)
                else:
                    nc.vector.tensor_copy(dst, pair)

            # ---- v bank (ni, h): v_s[ni, h] = A_s @ u_s + w1p_s
            def v_bank(at_t, st, v, ni, h):
                pv = ps_pool.tile([128, 512], F32, name="ps_v", tag="ps")
                for mi in range(NCH):
                    nc.tensor.matmul(
                        pv[:],
                        at_t[:, mi, ni * 128 : (ni + 1) * 128],
                        st[:, mi, 4 * h : 4 * h + 4, 128:256],
                        start=(mi == 0),
                        stop=(mi == NCH - 1),
                    )
                nc.vector.tensor_add(
                    v[:, ni, h * 512 : (h + 1) * 512],
                    pv[:],
                    st[:, ni, 4 * h : 4 * h + 4, 0:128],
                )

            # ---- fin bank (ni, h):
            #   s=0: fin[ni, h] = (A_0 @ v_0 + x0 @ What) + bias
            #        (the per-batch What matmuls accumulate into the same
            #        PSUM group; 128-wide, LDWEIGHTS hides under streaming)
            #   s=1: fin[ni, h] += A_1 @ v_1; then DMA out
            def fin_bank(s, at_t, v, ni, h, dma_split=1):
                pf = ps_pool.tile([128, 512], F32, name="ps_f", tag="ps")
                for mi in range(NCH):
                    nc.tensor.matmul(
                        pf[:],
                        at_t[:, mi, ni * 128 : (ni + 1) * 128],
                        v[:, mi, h * 512 : (h + 1) * 512],
                        start=(mi == 0),
                        stop=(s == 1 and mi == NCH - 1),
                    )
                if s == 0:
                    for bb in range(4):
                        nc.tensor.matmul(
                            pf[:, bb * 128 : (bb + 1) * 128],
                            x0t_t[:, 4 * h + bb, ni * 128 : (ni + 1) * 128],
                            wc[:, 0, :],
                            start=False,
                            stop=(bb == 3),
                        )
                fslc = fins[ni][:, h * 512 : (h + 1) * 512]
                w_ = 512 // dma_split
                for p in range(dma_split):
                    sl = slice(h * 512 + p * w_, h * 512 + (p + 1) * w_)
                    psl = slice(p * w_, (p + 1) * w_)
                    if s == 0:
                        nc.vector.tensor_add(
                            fins[ni][:, sl], pf[:, psl], b1024[:, sl]
                        )
                    else:
                        nc.vector.tensor_add(
                            fins[ni][:, sl], fins[ni][:, sl], pf[:, psl]
                        )
                        # out kicks on the scalar queue: the sync sequencer's
                        # 606ns DIRECT2D kicks would backlog the output drain
                        nc.scalar.dma_start(
                            out_d[
                                ni * 128 : (ni + 1) * 128,
                                4 * h + p * 4 // dma_split : 4 * h
                                + (p + 1) * 4 // dma_split,
                                :,
                            ],
                            fins[ni][:, sl],
                        )

            # ---- schedule (software-pipelined emission) ----
            # S0a half 0 (b 0-3): copy-paced (~212ns/step across DVE+ACT)
            # while x0t/at0 stream in; everything else is PE-bound, so the
            # remaining Sa work interleaves under the A-mult phases.
            for b in range(4):
                for mi in range(NCH):
                    sa_step(0, st0, b, mi)
                # kick x0t b+4 once b's steps are emitted: keeps at most a
                # few transfers outstanding so completion stays ~FIFO
                nc.sync.dma_start(x0t_t[:, b + 4, :], x0t_d[b + 4])
            # v0 h=0 banks interleaved with S0a half 1 (spreads copies under PE)
            for ni in range(NCH):
                v_bank(at_t0, st0, v0, ni, 0)
                for mi in range(NCH // 2):
                    sa_step(0, st0, 4 + ni // 2, (ni % 2) * 4 + mi)
            # v0 h=1 banks interleaved with first half of S1a (b 0-3)
            s1_steps = iter([(b, mi) for b in range(BL) for mi in range(NCH)])
            for ni in range(NCH):
                v_bank(at_t0, st0, v0, ni, 1)
                for _ in range(4):
                    b_, mi_ = next(s1_steps)
                    sa_step(1, st1, b_, mi_)

            # fin0 with the rest of S1a packed into its first half (4 per bank)
            for ni in range(NCH):
                for h in range(2):
                    fin_bank(0, at_t0, v0, ni, h)
                    if ni < 4:
                        for _ in range(4):
                            b_, mi_ = next(s1_steps)
                            sa_step(1, st1, b_, mi_)
                if ni < 4:
                    # at1 trickles in under fin0 (2 chunks per ni)
                    for mi in (2 * ni, 2 * ni + 1):
                        nc.sync.dma_start(
                            at_t1[:, mi, :],
                            at_d[1, mi * 128 : (mi + 1) * 128, :],
                        )

            for ni in range(NCH):
                v_bank(at_t1, st1, v1, ni, 0)
            for ni in range(NCH):
                v_bank(at_t1, st1, v1, ni, 1)
            for ni in range(NCH):
                for h in range(2):
                    # final bank: drain+DMA in halves to shorten the tail
                    last = ni == NCH - 1 and h == 1
                    fin_bank(1, at_t1, v1, ni, h, dma_split=2 if last else 1)

    nc.compile()
    _CACHE["nc"] = nc
    return nc


def kernel(supports, inputs, state, weight, biases, output_size, _trace=False):
    supports = np.asarray(supports, dtype=np.float32)
    inputs = np.asarray(inputs, dtype=np.float32)
    state = np.asarray(state, dtype=np.float32)
    weight = np.asarray(weight, dtype=np.float32)
    biases = np.asarray(biases, dtype=np.float32)
    O_ = int(output_size)
    assert O_ == O and inputs.shape == (B, N * 64) and supports.shape == (2, N, N)

    nc = _build()

    # host staging (layout + fp16 cast): A^T, x0^T, prepped W, tiled bias row
    at_np = np.ascontiguousarray(supports.transpose(0, 2, 1)).astype(np.float16)
    x0 = np.concatenate(
        [inputs.reshape(B, N, 64), state.reshape(B, N, 64)], axis=2
    )  # [B, N, F]
    x0t = x0.transpose(0, 2, 1)  # [B, F, N] view; per-core slice made contiguous
    wk = weight.reshape(F, 5, O)
    wprep = np.stack(
        [
            wk[:, 0] - wk[:, 2] - wk[:, 4],  # What
            wk[:, 1],
            2.0 * wk[:, 2],
            wk[:, 3],
            2.0 * wk[:, 4],
        ],
        axis=1,
    )
    w16 = np.ascontiguousarray(wprep).astype(np.float16)  # [F, 5, O]
    brow = np.ascontiguousarray(np.tile(biases, BL)[None, :]).astype(np.float32)

    in_maps = []
    for c in range(NCORES):
        in_maps.append(
            {
                "x0t": np.ascontiguousarray(
                    x0t[c * BL : (c + 1) * BL]
                ).astype(np.float16),
                "at": at_np,
                "w": w16,
                "b": brow,
            }
        )

    res = run_bass_kernel_spmd(
        nc, in_maps, core_ids=list(range(NCORES)), trace=_trace
    )
    kernel.last_result = res

    # out per core: [N, BL, O] -> full [B, N*O]
    parts = [res.results[c]["out"] for c in range(NCORES)]
    full = np.concatenate(parts, axis=1)  # [N, B, O]
    return np.ascontiguousarray(full.transpose(1, 0, 2)).reshape(B, N * O_)


# revision 32
# speedup vs baseline: 1.0329x; 1.0050x over previous
"""DiffusionGraphConv Trainium2 kernel (fp16 matmul pipeline).

Math (per batch b, support s, A = supports[s]):
  x0 = concat(inputs, state)                      # [N, F=128]
  reference out = sum_k x_k @ W_k  (+bias), k in {x0, x1_s0, x2_s0, x1_s1, x2_s1}
  with x1 = A x0, x2 = 2 A A x0 - x0, W_k = weight[f*5+k, :].

Restructured to avoid any on-chip transposes:
  out = x0 @ What + bias + sum_s A_s @ (x0 @ W1_s + A_s @ (x0 @ (2*W2_s)))
  with What = W_0 - W_2 - W_4, (W1_s, W2_s) = (W_1, W_2) for s=0, (W_3, W_4) for s=1.

All matmul operands are fp16 (PSUM accumulation stays fp32): fp16 streams
1 col/cycle like f32r, but its 128x128 stationary load goes through FWL and
hides under the previous matmul's streaming -- ~216 ns per 512-wide matmul
instead of f32r's ~273 ns (f32r self-loads its 4-byte weights serially).
fp16 also halves input DMA bytes, so both supports load upfront.
End-to-end quantization error ~5e-4 (fp32 accumulate, fp16 operands).

Layouts (per core, batch-sharded B_local = 8):
  x0T  DRAM [b=8, F=128, m=1024]   (host-staged transpose; lhsT tiles for x0@W)
  atT  DRAM [s=2, m=1024, n=1024]  (host-staged A^T; lhsT tiles for A-mults)
  All A-mult operands keep the node index on partitions -> layout-consistent
  chain, final out written per node-chunk as [n, b, o] blocks.
"""

import sys as _sys
import types as _types

try:
    import antenv.axon_hooks  # noqa: F401
except Exception:
    try:
        import antenv as _antenv

        _m = _types.ModuleType("antenv.axon_hooks")
        _m._hook = None
        _m.set_axon_ntff_profile_hook = lambda h: setattr(_m, "_hook", h)
        _m.get_axon_ntff_profile_hook = lambda: _m._hook
        _sys.modules["antenv.axon_hooks"] = _m
        _antenv.axon_hooks = _m
    except Exception:
        pass

import numpy as np

import concourse.mybir as mybir
import concourse.tile as tile
from concourse import bacc
from concourse.bass_utils import run_bass_kernel_spmd

NCORES = 8
B = 64
BL = B // NCORES  # 8 batches per core
N = 1024
F = 128
O = 128
NCH = N // 128  # 8 node chunks

F16 = mybir.dt.float16
F32 = mybir.dt.float32

WARMUP = 16

_CACHE = {}


def _build():
    if "nc" in _CACHE:
        return _CACHE["nc"]

    nc = bacc.Bacc(trn_type="TRN2", num_devices=NCORES, debug=False)

    x0t_d = nc.dram_tensor("x0t", [BL, F, N], F16, kind="ExternalInput")
    at_d = nc.dram_tensor("at", [2, N, N], F16, kind="ExternalInput")
    # host-prepped: [:,0]=What=W0-W2-W4, [:,1]=W1, [:,2]=2*W2, [:,3]=W3, [:,4]=2*W4
    w_d = nc.dram_tensor("w", [F, 5, O], F16, kind="ExternalInput")
    b_d = nc.dram_tensor("b", [1, BL * O], F32, kind="ExternalInput")  # tiled bias
    out_d = nc.dram_tensor("out", [N, BL, O], F32, kind="ExternalOutput")

    with tile.TileContext(nc) as tc:
        with (
            tc.tile_pool(name="big", bufs=1) as big,
            tc.tile_pool(name="small", bufs=1) as small,
            tc.tile_pool(name="ps_pool", bufs=8, space="PSUM") as ps_pool,
        ):
            # ---- persistent tiles ----
            # wc[:, k, :] = W_k; after prep: k=0 slot -> What, k=2/4 -> 2*W2/2*W4
            wc = small.tile([F, 5, O], F16)
            bias_t = small.tile([1, BL * O], F32)
            b1024 = small.tile([128, BL * O], F32)
            x0t_t = big.tile([F, BL, N], F16)  # 16KB/part
            at_t0 = big.tile([128, NCH, N], F16)  # 16KB/part
            at_t1 = big.tile([128, NCH, N], F16)  # 16KB/part
            st0 = big.tile([128, NCH, BL, 256], F16)  # 32KB/part
            st1 = big.tile([128, NCH, BL, 256], F16)  # 32KB/part
            v0 = big.tile([128, NCH, N], F16)  # 16KB/part
            v1 = big.tile([128, NCH, N], F16)  # 16KB/part
            fins = [
                big.tile([128, N], F32, name=f"fin{ni}") for ni in range(NCH)
            ]  # 32KB/part total

            # ---- PE warm-up: dummy matmuls during the DMA head so HAM
            # un-throttles (1.2 -> 2.4 GHz) before real work starts
            dummy = small.tile([128, 256], F16)
            dsink = small.tile([128, 1], F32)
            nc.vector.memset(dummy[:], 0.0)
            for _ in range(WARMUP):
                pw = ps_pool.tile([128, 256], F32, name="ps_w", tag="ps")
                nc.tensor.matmul(
                    pw[:], dummy[:, 0:128], dummy[:], start=True, stop=True
                )
            nc.vector.tensor_copy(dsink[:], pw[:, 0:1])

            # ---- head input DMAs. The DMA engines round-robin descriptors
            # across ALL outstanding transfers, so everything kicked at once
            # completes together near the end of the transfer window. Kick
            # only what the head needs (wc, x0t b0-3, at0); x0t b4-7 and at1
            # are kicked later, interleaved with the schedule (below).
            # per-transfer kicks (~606ns each, cost scales with line count --
            # consolidated multi-MB kicks stall everything queued behind
            # them, so keep transfers chunked). wc/bias ride the idle scalar
            # ring; x0t/at0 interleave on sync so at0 starts draining early.
            nc.scalar.dma_start(wc[:], w_d[:])
            nc.scalar.dma_start(bias_t[:], b_d[:])
            at0_kick = iter(range(NCH))
            for b in range(4):
                nc.sync.dma_start(x0t_t[:, b, :], x0t_d[b])
                for mi in ([next(at0_kick)] if b < 3 else list(at0_kick)):
                    nc.sync.dma_start(
                        at_t0[:, mi, :], at_d[0, mi * 128 : (mi + 1) * 128, :]
                    )

            # bias broadcast: first consumed by fin0's adds (~40us in)
            nc.gpsimd.partition_broadcast(b1024[:], bias_t[:])

            # ---- Sa step (b, mi): one stationary x0T tile, stream a W pair:
            #   s=0: [W1|2*W2], s=1: [W3|2*W4]; pair -> staging (one cast).
            # The x0@What term is folded into fin0's PSUM groups instead, so
            # the head has no bias/fins dependency and PSUM recycles at cast
            # pace. copies alternate DVE/ACT.
            def sa_step(s, st, b, mi):
                cnt = b * NCH + mi
                ps = ps_pool.tile([128, 512], F32, name="ps_sa", tag="ps")
                nc.tensor.matmul(
                    ps[:, :256],
                    x0t_t[:, b, mi * 128 : (mi + 1) * 128],
                    wc[:, 1:3, :] if s == 0 else wc[:, 3:5, :],
                    start=True,
                    stop=True,
                )
                pair = ps[:, 0:256]
                dst = st[:, mi, b, :]
                flip = (cnt % 8) < 5 if s == 1 else cnt % 2 == 0
                if flip:
                    nc.scalar.copy(dst, pair)
                else:
                    nc.vector.tensor_copy(dst, pair)

            # ---- v bank (ni, h): v_s[ni, h] = A_s @ u_s + w1p_s
            # mid=callback emitted between mi 0-3 and 4-7 (lets the first
            # banks start on at0's landed half while its tail streams in)
            def v_bank(at_t, st, v, ni, h, mid=None):
                pv = ps_pool.tile([128, 512], F32, name="ps_v", tag="ps")
                for mi in range(NCH):
                    if mi == 4 and mid is not None:
                        mid()
                    nc.tensor.matmul(
                        pv[:],
                        at_t[:, mi, ni * 128 : (ni + 1) * 128],
                        st[:, mi, 4 * h : 4 * h + 4, 128:256],
                        start=(mi == 0),
                        stop=(mi == NCH - 1),
                    )
                nc.vector.tensor_add(
                    v[:, ni, h * 512 : (h + 1) * 512],
                    pv[:],
                    st[:, ni, 4 * h : 4 * h + 4, 0:128],
                )

            # ---- fin bank (ni, h):
            #   s=0: fin[ni, h] = (A_0 @ v_0 + x0 @ What) + bias
            #        (the per-batch What matmuls accumulate into the same
            #        PSUM group; 128-wide, LDWEIGHTS hides under streaming)
            #   s=1: fin[ni, h] += A_1 @ v_1; then DMA out
            def fin_bank(s, at_t, v, ni, h, dma_split=1):
                pf = ps_pool.tile([128, 512], F32, name="ps_f", tag="ps")
                for mi in range(NCH):
                    nc.tensor.matmul(
                        pf[:],
                        at_t[:, mi, ni * 128 : (ni + 1) * 128],
                        v[:, mi, h * 512 : (h + 1) * 512],
                        start=(mi == 0),
                        stop=(s == 1 and mi == NCH - 1),
                    )
                if s == 0:
                    for bb in range(4):
                        nc.tensor.matmul(
                            pf[:, bb * 128 : (bb + 1) * 128],
                            x0t_t[:, 4 * h + bb, ni * 128 : (ni + 1) * 128],
                            wc[:, 0, :],
                            start=False,
                            stop=(bb == 3),
                        )
                fslc = fins[ni][:, h * 512 : (h + 1) * 512]
                w_ = 512 // dma_split
                for p in range(dma_split):
                    sl = slice(h * 512 + p * w_, h * 512 + (p + 1) * w_)
                    psl = slice(p * w_, (p + 1) * w_)
                    if s == 0:
                        nc.vector.tensor_add(
                            fins[ni][:, sl], pf[:, psl], b1024[:, sl]
                        )
                    else:
                        nc.vector.tensor_add(
                            fins[ni][:, sl], fins[ni][:, sl], pf[:, psl]
                        )
                        # out kicks on the scalar queue: the sync sequencer's
                        # 606ns DIRECT2D kicks would backlog the output drain
                        nc.scalar.dma_start(
                            out_d[
                                ni * 128 : (ni + 1) * 128,
                                4 * h + p * 4 // dma_split : 4 * h
                                + (p + 1) * 4 // dma_split,
                                :,
                            ],
                            fins[ni][:, sl],
                        )

            # ---- schedule (software-pipelined emission) ----
            # S0a half 0 (b 0-3): copy-paced (~212ns/step across DVE+ACT)
            # while x0t/at0 stream in; everything else is PE-bound, so the
            # remaining Sa work interleaves under the A-mult phases.
            for b in range(4):
                for mi in range(NCH):
                    sa_step(0, st0, b, mi)
                # kick x0t b+4 once b's steps are emitted: keeps at most a
                # few transfers outstanding so completion stays ~FIFO
                nc.sync.dma_start(x0t_t[:, b + 4, :], x0t_d[b + 4])
            # v0 h=0 banks interleaved with S0a half 1 (spreads copies under PE)
            for ni in range(NCH):
                def _sa4(ni=ni):
                    for mi in range(NCH // 2):
                        sa_step(0, st0, 4 + ni // 2, (ni % 2) * 4 + mi)
                if ni < 2:
                    v_bank(at_t0, st0, v0, ni, 0, mid=_sa4)
                else:
                    v_bank(at_t0, st0, v0, ni, 0)
                    _sa4()
            # v0 h=1 banks interleaved with first half of S1a (b 0-3)
            s1_steps = iter([(b, mi) for b in range(BL) for mi in range(NCH)])
            for ni in range(NCH):
                v_bank(at_t0, st0, v0, ni, 1)
                for _ in range(4):
                    b_, mi_ = next(s1_steps)
                    sa_step(1, st1, b_, mi_)

            # fin0 with the rest of S1a packed into its first half (4 per bank)
            for ni in range(NCH):
                for h in range(2):
                    fin_bank(0, at_t0, v0, ni, h)
                    if ni < 4:
                        for _ in range(4):
                            b_, mi_ = next(s1_steps)
                            sa_step(1, st1, b_, mi_)
                if ni < 4:
                    # at1 trickles in under fin0 (2 chunks per ni)
                    for mi in (2 * ni, 2 * ni + 1):
                        nc.sync.dma_start(
                            at_t1[:, mi, :],
                            at_d[1, mi * 128 : (mi + 1) * 128, :],
                        )

            for ni in range(NCH):
                v_bank(at_t1, st1, v1, ni, 0)
            for ni in range(NCH):
                v_bank(at_t1, st1, v1, ni, 1)
            for ni in range(NCH):
                for h in range(2):
                    # final bank: drain+DMA in halves to shorten the tail
                    last = ni == NCH - 1 and h == 1
                    fin_bank(1, at_t1, v1, ni, h, dma_split=2 if last else 1)

    nc.compile()
    _CACHE["nc"] = nc
    return nc


def kernel(supports, inputs, state, weight, biases, output_size, _trace=False):
    supports = np.asarray(supports, dtype=np.float32)
    inputs = np.asarray(inputs, dtype=np.float32)
    state = np.asarray(state, dtype=np.float32)
    weight = np.asarray(weight, dtype=np.float32)
    biases = np.asarray(biases, dtype=np.float32)
    O_ = int(output_size)
    assert O_ == O and inputs.shape == (B, N * 64) and supports.shape == (2, N, N)

    nc = _build()

    # host staging (layout + fp16 cast): A^T, x0^T, prepped W, tiled bias row
    at_np = np.ascontiguousarray(supports.transpose(0, 2, 1)).astype(np.float16)
    x0 = np.concatenate(
        [inputs.reshape(B, N, 64), state.reshape(B, N, 64)], axis=2
    )  # [B, N, F]
    x0t = x0.transpose(0, 2, 1)  # [B, F, N] view; per-core slice made contiguous
    wk = weight.reshape(F, 5, O)
    wprep = np.stack(
        [
            wk[:, 0] - wk[:, 2] - wk[:, 4],  # What
            wk[:, 1],
            2.0 * wk[:, 2],
            wk[:, 3],
            2.0 * wk[:, 4],
        ],
        axis=1,
    )
    w16 = np.ascontiguousarray(wprep).astype(np.float16)  # [F, 5, O]
    brow = np.ascontiguousarray(np.tile(biases, BL)[None, :]).astype(np.float32)

    in_maps = []
    for c in range(NCORES):
        in_maps.append(
            {
                "x0t": np.ascontiguousarray(
                    x0t[c * BL : (c + 1) * BL]
                ).astype(np.float16),
                "at": at_np,
                "w": w16,
                "b": brow,
            }
        )

    res = run_bass_kernel_spmd(
        nc, in_maps, core_ids=list(range(NCORES)), trace=_trace
    )
    kernel.last_result = res

    # out per core: [N, BL, O] -> full [B, N*O]
    parts = [res.results[c]["out"] for c in range(NCORES)]
    full = np.concatenate(parts, axis=1)  # [N, B, O]
    return np.ascontiguousarray(full.transpose(1, 0, 2)).reshape(B, N * O_)


# revision 33
# speedup vs baseline: 1.0391x; 1.0060x over previous
"""DiffusionGraphConv Trainium2 kernel (fp16 matmul pipeline).

Math (per batch b, support s, A = supports[s]):
  x0 = concat(inputs, state)                      # [N, F=128]
  reference out = sum_k x_k @ W_k  (+bias), k in {x0, x1_s0, x2_s0, x1_s1, x2_s1}
  with x1 = A x0, x2 = 2 A A x0 - x0, W_k = weight[f*5+k, :].

Restructured to avoid any on-chip transposes:
  out = x0 @ What + bias + sum_s A_s @ (x0 @ W1_s + A_s @ (x0 @ (2*W2_s)))
  with What = W_0 - W_2 - W_4, (W1_s, W2_s) = (W_1, W_2) for s=0, (W_3, W_4) for s=1.

All matmul operands are fp16 (PSUM accumulation stays fp32): fp16 streams
1 col/cycle like f32r, but its 128x128 stationary load goes through FWL and
hides under the previous matmul's streaming -- ~216 ns per 512-wide matmul
instead of f32r's ~273 ns (f32r self-loads its 4-byte weights serially).
fp16 also halves input DMA bytes, so both supports load upfront.
End-to-end quantization error ~5e-4 (fp32 accumulate, fp16 operands).

Layouts (per core, batch-sharded B_local = 8):
  x0T  DRAM [b=8, F=128, m=1024]   (host-staged transpose; lhsT tiles for x0@W)
  atT  DRAM [s=2, m=1024, n=1024]  (host-staged A^T; lhsT tiles for A-mults)
  All A-mult operands keep the node index on partitions -> layout-consistent
  chain, final out written per node-chunk as [n, b, o] blocks.
"""

import sys as _sys
import types as _types

try:
    import antenv.axon_hooks  # noqa: F401
except Exception:
    try:
        import antenv as _antenv

        _m = _types.ModuleType("antenv.axon_hooks")
        _m._hook = None
        _m.set_axon_ntff_profile_hook = lambda h: setattr(_m, "_hook", h)
        _m.get_axon_ntff_profile_hook = lambda: _m._hook
        _sys.modules["antenv.axon_hooks"] = _m
        _antenv.axon_hooks = _m
    except Exception:
        pass

import numpy as np

import concourse.mybir as mybir
import concourse.tile as tile
from concourse import bacc
from concourse.bass_utils import run_bass_kernel_spmd

NCORES = 8
B = 64
BL = B // NCORES  # 8 batches per core
N = 1024
F = 128
O = 128
NCH = N // 128  # 8 node chunks

F16 = mybir.dt.float16
F32 = mybir.dt.float32

WARMUP = 16

_CACHE = {}


def _build():
    if "nc" in _CACHE:
        return _CACHE["nc"]

    nc = bacc.Bacc(trn_type="TRN2", num_devices=NCORES, debug=False)

    x0t_d = nc.dram_tensor("x0t", [BL, F, N], F16, kind="ExternalInput")
    at_d = nc.dram_tensor("at", [2, N, N], F16, kind="ExternalInput")
    # host-prepped: [:,0]=What=W0-W2-W4, [:,1]=W1, [:,2]=2*W2, [:,3]=W3, [:,4]=2*W4
    w_d = nc.dram_tensor("w", [F, 5, O], F16, kind="ExternalInput")
    b_d = nc.dram_tensor("b", [1, BL * O], F32, kind="ExternalInput")  # tiled bias
    out_d = nc.dram_tensor("out", [N, BL, O], F32, kind="ExternalOutput")

    with tile.TileContext(nc) as tc:
        with (
            tc.tile_pool(name="big", bufs=1) as big,
            tc.tile_pool(name="small", bufs=1) as small,
            tc.tile_pool(name="ps_pool", bufs=8, space="PSUM") as ps_pool,
        ):
            # ---- persistent tiles ----
            # wc[:, k, :] = W_k; after prep: k=0 slot -> What, k=2/4 -> 2*W2/2*W4
            wc = small.tile([F, 5, O], F16)
            bias_t = small.tile([1, BL * O], F32)
            b1024 = small.tile([128, BL * O], F32)
            x0t_t = big.tile([F, BL, N], F16)  # 16KB/part
            at_t0 = big.tile([128, NCH, N], F16)  # 16KB/part
            at_t1 = big.tile([128, NCH, N], F16)  # 16KB/part
            st0 = big.tile([128, NCH, BL, 256], F16)  # 32KB/part
            st1 = big.tile([128, NCH, BL, 256], F16)  # 32KB/part
            v0 = big.tile([128, NCH, N], F16)  # 16KB/part
            v1 = big.tile([128, NCH, N], F16)  # 16KB/part
            fins = [
                big.tile([128, N], F32, name=f"fin{ni}") for ni in range(NCH)
            ]  # 32KB/part total

            # ---- PE warm-up: dummy matmuls during the DMA head so HAM
            # un-throttles (1.2 -> 2.4 GHz) before real work starts
            dummy = small.tile([128, 256], F16)
            dsink = small.tile([128, 1], F32)
            nc.vector.memset(dummy[:], 0.0)
            for _ in range(WARMUP):
                pw = ps_pool.tile([128, 256], F32, name="ps_w", tag="ps")
                nc.tensor.matmul(
                    pw[:], dummy[:, 0:128], dummy[:], start=True, stop=True
                )
            nc.vector.tensor_copy(dsink[:], pw[:, 0:1])

            # ---- head input DMAs. The DMA engines round-robin descriptors
            # across ALL outstanding transfers, so everything kicked at once
            # completes together near the end of the transfer window. Kick
            # only what the head needs (wc, x0t b0-3, at0); x0t b4-7 and at1
            # are kicked later, interleaved with the schedule (below).
            # per-transfer kicks (~606ns each, cost scales with line count --
            # consolidated multi-MB kicks stall everything queued behind
            # them, so keep transfers chunked). wc/bias ride the idle scalar
            # ring; x0t/at0 interleave on sync so at0 starts draining early.
            nc.scalar.dma_start(wc[:], w_d[:])
            nc.scalar.dma_start(bias_t[:], b_d[:])
            for b in range(4):
                nc.sync.dma_start(x0t_t[:, b, :], x0t_d[b])
            # at0 kicks ride the otherwise-idle gpsimd ring so they issue in
            # parallel with the x0t kicks on sync
            for mi in range(NCH):
                nc.gpsimd.dma_start(
                    at_t0[:, mi, :], at_d[0, mi * 128 : (mi + 1) * 128, :]
                )

            # bias broadcast: first consumed by fin0's adds (~40us in)
            nc.gpsimd.partition_broadcast(b1024[:], bias_t[:])

            # ---- Sa step (b, mi): one stationary x0T tile, stream a W pair:
            #   s=0: [W1|2*W2], s=1: [W3|2*W4]; pair -> staging (one cast).
            # The x0@What term is folded into fin0's PSUM groups instead, so
            # the head has no bias/fins dependency and PSUM recycles at cast
            # pace. copies alternate DVE/ACT.
            def sa_step(s, st, b, mi):
                cnt = b * NCH + mi
                ps = ps_pool.tile([128, 512], F32, name="ps_sa", tag="ps")
                nc.tensor.matmul(
                    ps[:, :256],
                    x0t_t[:, b, mi * 128 : (mi + 1) * 128],
                    wc[:, 1:3, :] if s == 0 else wc[:, 3:5, :],
                    start=True,
                    stop=True,
                )
                pair = ps[:, 0:256]
                dst = st[:, mi, b, :]
                flip = (cnt % 8) < 5 if s == 1 else cnt % 2 == 0
                if flip:
                    nc.scalar.copy(dst, pair)
                else:
                    nc.vector.tensor_copy(dst, pair)

            # ---- v bank (ni, h): v_s[ni, h] = A_s @ u_s + w1p_s
            # mid=callback emitted between mi 0-3 and 4-7 (lets the first
            # banks start on at0's landed half while its tail streams in)
            def v_bank(at_t, st, v, ni, h, mid=None):
                pv = ps_pool.tile([128, 512], F32, name="ps_v", tag="ps")
                for mi in range(NCH):
                    if mi == 4 and mid is not None:
                        mid()
                    nc.tensor.matmul(
                        pv[:],
                        at_t[:, mi, ni * 128 : (ni + 1) * 128],
                        st[:, mi, 4 * h : 4 * h + 4, 128:256],
                        start=(mi == 0),
                        stop=(mi == NCH - 1),
                    )
                nc.vector.tensor_add(
                    v[:, ni, h * 512 : (h + 1) * 512],
                    pv[:],
                    st[:, ni, 4 * h : 4 * h + 4, 0:128],
                )

            # ---- fin bank (ni, h):
            #   s=0: fin[ni, h] = (A_0 @ v_0 + x0 @ What) + bias
            #        (the per-batch What matmuls accumulate into the same
            #        PSUM group; 128-wide, LDWEIGHTS hides under streaming)
            #   s=1: fin[ni, h] += A_1 @ v_1; then DMA out
            def fin_bank(s, at_t, v, ni, h, dma_split=1):
                pf = ps_pool.tile([128, 512], F32, name="ps_f", tag="ps")
                for mi in range(NCH):
                    nc.tensor.matmul(
                        pf[:],
                        at_t[:, mi, ni * 128 : (ni + 1) * 128],
                        v[:, mi, h * 512 : (h + 1) * 512],
                        start=(mi == 0),
                        stop=(s == 1 and mi == NCH - 1),
                    )
                if s == 0:
                    for bb in range(4):
                        nc.tensor.matmul(
                            pf[:, bb * 128 : (bb + 1) * 128],
                            x0t_t[:, 4 * h + bb, ni * 128 : (ni + 1) * 128],
                            wc[:, 0, :],
                            start=False,
                            stop=(bb == 3),
                        )
                fslc = fins[ni][:, h * 512 : (h + 1) * 512]
                w_ = 512 // dma_split
                for p in range(dma_split):
                    sl = slice(h * 512 + p * w_, h * 512 + (p + 1) * w_)
                    psl = slice(p * w_, (p + 1) * w_)
                    if s == 0:
                        nc.vector.tensor_add(
                            fins[ni][:, sl], pf[:, psl], b1024[:, sl]
                        )
                    else:
                        nc.vector.tensor_add(
                            fins[ni][:, sl], fins[ni][:, sl], pf[:, psl]
                        )
                        # out kicks on the scalar queue: the sync sequencer's
                        # 606ns DIRECT2D kicks would backlog the output drain
                        nc.scalar.dma_start(
                            out_d[
                                ni * 128 : (ni + 1) * 128,
                                4 * h + p * 4 // dma_split : 4 * h
                                + (p + 1) * 4 // dma_split,
                                :,
                            ],
                            fins[ni][:, sl],
                        )

            # ---- schedule (software-pipelined emission) ----
            # S0a half 0 (b 0-3): copy-paced (~212ns/step across DVE+ACT)
            # while x0t/at0 stream in; everything else is PE-bound, so the
            # remaining Sa work interleaves under the A-mult phases.
            for b in range(4):
                for mi in range(NCH):
                    sa_step(0, st0, b, mi)
                # kick x0t b+4 once b's steps are emitted: keeps at most a
                # few transfers outstanding so completion stays ~FIFO
                nc.sync.dma_start(x0t_t[:, b + 4, :], x0t_d[b + 4])
            # v0 h=0 banks interleaved with S0a half 1 (spreads copies under PE)
            for ni in range(NCH):
                def _sa4(ni=ni):
                    for mi in range(NCH // 2):
                        sa_step(0, st0, 4 + ni // 2, (ni % 2) * 4 + mi)
                if ni < 2:
                    v_bank(at_t0, st0, v0, ni, 0, mid=_sa4)
                else:
                    v_bank(at_t0, st0, v0, ni, 0)
                    _sa4()
            # v0 h=1 banks interleaved with first half of S1a (b 0-3)
            s1_steps = iter([(b, mi) for b in range(BL) for mi in range(NCH)])
            for ni in range(NCH):
                v_bank(at_t0, st0, v0, ni, 1)
                for _ in range(4):
                    b_, mi_ = next(s1_steps)
                    sa_step(1, st1, b_, mi_)

            # fin0 with the rest of S1a packed into its first half (4 per bank)
            for ni in range(NCH):
                for h in range(2):
                    fin_bank(0, at_t0, v0, ni, h)
                    if ni < 4:
                        for _ in range(4):
                            b_, mi_ = next(s1_steps)
                            sa_step(1, st1, b_, mi_)
                if ni < 4:
                    # at1 trickles in under fin0 (2 chunks per ni)
                    for mi in (2 * ni, 2 * ni + 1):
                        nc.sync.dma_start(
                            at_t1[:, mi, :],
                            at_d[1, mi * 128 : (mi + 1) * 128, :],
                        )

            for ni in range(NCH):
                v_bank(at_t1, st1, v1, ni, 0)
            for ni in range(NCH):
                v_bank(at_t1, st1, v1, ni, 1)
            for ni in range(NCH):
                for h in range(2):
                    # final bank: drain+DMA in halves to shorten the tail
                    last = ni == NCH - 1 and h == 1
                    fin_bank(1, at_t1, v1, ni, h, dma_split=2 if last else 1)

    nc.compile()
    _CACHE["nc"] = nc
    return nc


def kernel(supports, inputs, state, weight, biases, output_size, _trace=False):
    supports = np.asarray(supports, dtype=np.float32)
    inputs = np.asarray(inputs, dtype=np.float32)
    state = np.asarray(state, dtype=np.float32)
    weight = np.asarray(weight, dtype=np.float32)
    biases = np.asarray(biases, dtype=np.float32)
    O_ = int(output_size)
    assert O_ == O and inputs.shape == (B, N * 64) and supports.shape == (2, N, N)

    nc = _build()

    # host staging (layout + fp16 cast): A^T, x0^T, prepped W, tiled bias row
    at_np = np.ascontiguousarray(supports.transpose(0, 2, 1)).astype(np.float16)
    x0 = np.concatenate(
        [inputs.reshape(B, N, 64), state.reshape(B, N, 64)], axis=2
    )  # [B, N, F]
    x0t = x0.transpose(0, 2, 1)  # [B, F, N] view; per-core slice made contiguous
    wk = weight.reshape(F, 5, O)
    wprep = np.stack(
        [
            wk[:, 0] - wk[:, 2] - wk[:, 4],  # What
            wk[:, 1],
            2.0 * wk[:, 2],
            wk[:, 3],
            2.0 * wk[:, 4],
        ],
        axis=1,
    )
    w16 = np.ascontiguousarray(wprep).astype(np.float16)  # [F, 5, O]
    brow = np.ascontiguousarray(np.tile(biases, BL)[None, :]).astype(np.float32)

    in_maps = []
    for c in range(NCORES):
        in_maps.append(
            {
                "x0t": np.ascontiguousarray(
                    x0t[c * BL : (c + 1) * BL]
                ).astype(np.float16),
                "at": at_np,
                "w": w16,
                "b": brow,
            }
        )

    res = run_bass_kernel_spmd(
        nc, in_maps, core_ids=list(range(NCORES)), trace=_trace
    )
    kernel.last_result = res

    # out per core: [N, BL, O] -> full [B, N*O]
    parts = [res.results[c]["out"] for c in range(NCORES)]
    full = np.concatenate(parts, axis=1)  # [N, B, O]
    return np.ascontiguousarray(full.transpose(1, 0, 2)).reshape(B, N * O_)


# revision 34
# speedup vs baseline: 1.0411x; 1.0019x over previous
"""DiffusionGraphConv Trainium2 kernel (fp16 matmul pipeline).

Math (per batch b, support s, A = supports[s]):
  x0 = concat(inputs, state)                      # [N, F=128]
  reference out = sum_k x_k @ W_k  (+bias), k in {x0, x1_s0, x2_s0, x1_s1, x2_s1}
  with x1 = A x0, x2 = 2 A A x0 - x0, W_k = weight[f*5+k, :].

Restructured to avoid any on-chip transposes:
  out = x0 @ What + bias + sum_s A_s @ (x0 @ W1_s + A_s @ (x0 @ (2*W2_s)))
  with What = W_0 - W_2 - W_4, (W1_s, W2_s) = (W_1, W_2) for s=0, (W_3, W_4) for s=1.

All matmul operands are fp16 (PSUM accumulation stays fp32): fp16 streams
1 col/cycle like f32r, but its 128x128 stationary load goes through FWL and
hides under the previous matmul's streaming -- ~216 ns per 512-wide matmul
instead of f32r's ~273 ns (f32r self-loads its 4-byte weights serially).
fp16 also halves input DMA bytes, so both supports load upfront.
End-to-end quantization error ~5e-4 (fp32 accumulate, fp16 operands).

Layouts (per core, batch-sharded B_local = 8):
  x0T  DRAM [b=8, F=128, m=1024]   (host-staged transpose; lhsT tiles for x0@W)
  atT  DRAM [s=2, m=1024, n=1024]  (host-staged A^T; lhsT tiles for A-mults)
  All A-mult operands keep the node index on partitions -> layout-consistent
  chain, final out written per node-chunk as [n, b, o] blocks.
"""

import sys as _sys
import types as _types

try:
    import antenv.axon_hooks  # noqa: F401
except Exception:
    try:
        import antenv as _antenv

        _m = _types.ModuleType("antenv.axon_hooks")
        _m._hook = None
        _m.set_axon_ntff_profile_hook = lambda h: setattr(_m, "_hook", h)
        _m.get_axon_ntff_profile_hook = lambda: _m._hook
        _sys.modules["antenv.axon_hooks"] = _m
        _antenv.axon_hooks = _m
    except Exception:
        pass

import numpy as np

import concourse.mybir as mybir
import concourse.tile as tile
from concourse import bacc
from concourse.bass_utils import run_bass_kernel_spmd

NCORES = 8
B = 64
BL = B // NCORES  # 8 batches per core
N = 1024
F = 128
O = 128
NCH = N // 128  # 8 node chunks

F16 = mybir.dt.float16
F32 = mybir.dt.float32

WARMUP = 16

_CACHE = {}


def _build():
    if "nc" in _CACHE:
        return _CACHE["nc"]

    nc = bacc.Bacc(trn_type="TRN2", num_devices=NCORES, debug=False)

    x0t_d = nc.dram_tensor("x0t", [BL, F, N], F16, kind="ExternalInput")
    at_d = nc.dram_tensor("at", [2, N, N], F16, kind="ExternalInput")
    # host-prepped: [:,0]=What=W0-W2-W4, [:,1]=W1, [:,2]=2*W2, [:,3]=W3, [:,4]=2*W4
    w_d = nc.dram_tensor("w", [F, 5, O], F16, kind="ExternalInput")
    b_d = nc.dram_tensor("b", [1, BL * O], F32, kind="ExternalInput")  # tiled bias
    out_d = nc.dram_tensor("out", [N, BL, O], F32, kind="ExternalOutput")

    with tile.TileContext(nc) as tc:
        with (
            tc.tile_pool(name="big", bufs=1) as big,
            tc.tile_pool(name="small", bufs=1) as small,
            tc.tile_pool(name="ps_pool", bufs=8, space="PSUM") as ps_pool,
        ):
            # ---- persistent tiles ----
            # wc[:, k, :] = W_k; after prep: k=0 slot -> What, k=2/4 -> 2*W2/2*W4
            wc = small.tile([F, 5, O], F16)
            bias_t = small.tile([1, BL * O], F32)
            b1024 = small.tile([128, BL * O], F32)
            x0t_t = big.tile([F, BL, N], F16)  # 16KB/part
            at_t0 = big.tile([128, NCH, N], F16)  # 16KB/part
            at_t1 = big.tile([128, NCH, N], F16)  # 16KB/part
            st0 = big.tile([128, NCH, BL, 256], F16)  # 32KB/part
            st1 = big.tile([128, NCH, BL, 256], F16)  # 32KB/part
            v0 = big.tile([128, NCH, N], F16)  # 16KB/part
            v1 = big.tile([128, NCH, N], F16)  # 16KB/part
            fins = [
                big.tile([128, N], F32, name=f"fin{ni}") for ni in range(NCH)
            ]  # 32KB/part total

            # ---- PE warm-up: dummy matmuls during the DMA head so HAM
            # un-throttles (1.2 -> 2.4 GHz) before real work starts
            dummy = small.tile([128, 256], F16)
            dsink = small.tile([128, 1], F32)
            nc.vector.memset(dummy[:], 0.0)
            for _ in range(WARMUP):
                pw = ps_pool.tile([128, 256], F32, name="ps_w", tag="ps")
                nc.tensor.matmul(
                    pw[:], dummy[:, 0:128], dummy[:], start=True, stop=True
                )
            nc.vector.tensor_copy(dsink[:], pw[:, 0:1])

            # ---- head input DMAs. The DMA engines round-robin descriptors
            # across ALL outstanding transfers, so everything kicked at once
            # completes together near the end of the transfer window. Kick
            # only what the head needs (wc, x0t b0-3, at0); x0t b4-7 and at1
            # are kicked later, interleaved with the schedule (below).
            # per-transfer kicks (~606ns each, cost scales with line count --
            # consolidated multi-MB kicks stall everything queued behind
            # them, so keep transfers chunked). wc/bias ride the idle scalar
            # ring; x0t/at0 interleave on sync so at0 starts draining early.
            nc.scalar.dma_start(wc[:], w_d[:])
            nc.scalar.dma_start(bias_t[:], b_d[:])
            for b in range(4):
                nc.sync.dma_start(x0t_t[:, b, :], x0t_d[b])
            # at0 kicks ride the otherwise-idle gpsimd ring so they issue in
            # parallel with the x0t kicks on sync
            for mi in range(NCH):
                nc.gpsimd.dma_start(
                    at_t0[:, mi, :], at_d[0, mi * 128 : (mi + 1) * 128, :]
                )

            # bias broadcast: first consumed by fin0's adds (~40us in)
            nc.gpsimd.partition_broadcast(b1024[:], bias_t[:])

            # ---- Sa step (b, mi): one stationary x0T tile, stream a W pair:
            #   s=0: [W1|2*W2], s=1: [W3|2*W4]; pair -> staging (one cast).
            # The x0@What term is folded into fin0's PSUM groups instead, so
            # the head has no bias/fins dependency and PSUM recycles at cast
            # pace. copies alternate DVE/ACT.
            def sa_step(s, st, b, mi):
                cnt = b * NCH + mi
                ps = ps_pool.tile([128, 512], F32, name="ps_sa", tag="ps")
                nc.tensor.matmul(
                    ps[:, :256],
                    x0t_t[:, b, mi * 128 : (mi + 1) * 128],
                    wc[:, 1:3, :] if s == 0 else wc[:, 3:5, :],
                    start=True,
                    stop=True,
                )
                pair = ps[:, 0:256]
                dst = st[:, mi, b, :]
                flip = (cnt % 8) < 5 if s == 1 else cnt % 2 == 0
                if flip:
                    nc.scalar.copy(dst, pair)
                else:
                    nc.vector.tensor_copy(dst, pair)

            # ---- v bank (ni, h): v_s[ni, h] = A_s @ u_s + w1p_s
            # mid=callback emitted between mi 0-3 and 4-7 (lets the first
            # banks start on at0's landed half while its tail streams in)
            def v_bank(at_t, st, v, ni, h, mid=None):
                pv = ps_pool.tile([128, 512], F32, name="ps_v", tag="ps")
                for mi in range(NCH):
                    if mi == 4 and mid is not None:
                        mid()
                    nc.tensor.matmul(
                        pv[:],
                        at_t[:, mi, ni * 128 : (ni + 1) * 128],
                        st[:, mi, 4 * h : 4 * h + 4, 128:256],
                        start=(mi == 0),
                        stop=(mi == NCH - 1),
                    )
                nc.vector.tensor_add(
                    v[:, ni, h * 512 : (h + 1) * 512],
                    pv[:],
                    st[:, ni, 4 * h : 4 * h + 4, 0:128],
                )

            # ---- fin bank (ni, h):
            #   s=0: fin[ni, h] = (A_0 @ v_0 + x0 @ What) + bias
            #        (the per-batch What matmuls accumulate into the same
            #        PSUM group; 128-wide, LDWEIGHTS hides under streaming)
            #   s=1: fin[ni, h] += A_1 @ v_1; then DMA out
            def fin_bank(s, at_t, v, ni, h, dma_split=1):
                pf = ps_pool.tile([128, 512], F32, name="ps_f", tag="ps")
                for mi in range(NCH):
                    nc.tensor.matmul(
                        pf[:],
                        at_t[:, mi, ni * 128 : (ni + 1) * 128],
                        v[:, mi, h * 512 : (h + 1) * 512],
                        start=(mi == 0),
                        stop=(s == 1 and mi == NCH - 1),
                    )
                if s == 0:
                    for bb in range(4):
                        nc.tensor.matmul(
                            pf[:, bb * 128 : (bb + 1) * 128],
                            x0t_t[:, 4 * h + bb, ni * 128 : (ni + 1) * 128],
                            wc[:, 0, :],
                            start=False,
                            stop=(bb == 3),
                        )
                fslc = fins[ni][:, h * 512 : (h + 1) * 512]
                w_ = 512 // dma_split
                for p in range(dma_split):
                    sl = slice(h * 512 + p * w_, h * 512 + (p + 1) * w_)
                    psl = slice(p * w_, (p + 1) * w_)
                    if s == 0:
                        nc.vector.tensor_add(
                            fins[ni][:, sl], pf[:, psl], b1024[:, sl]
                        )
                    else:
                        nc.vector.tensor_add(
                            fins[ni][:, sl], fins[ni][:, sl], pf[:, psl]
                        )
                        # out kicks on the scalar queue: the sync sequencer's
                        # 606ns DIRECT2D kicks would backlog the output drain
                        nc.scalar.dma_start(
                            out_d[
                                ni * 128 : (ni + 1) * 128,
                                4 * h + p * 4 // dma_split : 4 * h
                                + (p + 1) * 4 // dma_split,
                                :,
                            ],
                            fins[ni][:, sl],
                        )

            # ---- schedule (software-pipelined emission) ----
            # S0a half 0 (b 0-3): copy-paced (~212ns/step across DVE+ACT)
            # while x0t/at0 stream in; everything else is PE-bound, so the
            # remaining Sa work interleaves under the A-mult phases.
            for b in range(4):
                for mi in range(NCH):
                    sa_step(0, st0, b, mi)
                # kick x0t b+4 once b's steps are emitted: keeps at most a
                # few transfers outstanding so completion stays ~FIFO
                nc.sync.dma_start(x0t_t[:, b + 4, :], x0t_d[b + 4])
            # v0 h=0 banks interleaved with S0a half 1 (spreads copies under PE)
            for ni in range(NCH):
                def _sa4(ni=ni):
                    for mi in range(NCH // 2):
                        sa_step(0, st0, 4 + ni // 2, (ni % 2) * 4 + mi)
                if ni < 2:
                    v_bank(at_t0, st0, v0, ni, 0, mid=_sa4)
                else:
                    v_bank(at_t0, st0, v0, ni, 0)
                    _sa4()
            # v0 h=1 banks interleaved with first half of S1a (b 0-3)
            s1_steps = iter([(b, mi) for b in range(BL) for mi in range(NCH)])
            for ni in range(NCH):
                v_bank(at_t0, st0, v0, ni, 1)
                for _ in range(4):
                    b_, mi_ = next(s1_steps)
                    sa_step(1, st1, b_, mi_)

            # fin0 with the rest of S1a packed into its first half (4 per bank)
            for ni in range(NCH):
                for h in range(2):
                    fin_bank(0, at_t0, v0, ni, h)
                    if ni < 4:
                        for _ in range(4):
                            b_, mi_ = next(s1_steps)
                            sa_step(1, st1, b_, mi_)
                if ni < 4:
                    # at1 trickles in under fin0 (2 chunks per ni)
                    for mi in (2 * ni, 2 * ni + 1):
                        nc.sync.dma_start(
                            at_t1[:, mi, :],
                            at_d[1, mi * 128 : (mi + 1) * 128, :],
                        )

            for ni in range(NCH):
                v_bank(at_t1, st1, v1, ni, 0)
            for ni in range(NCH):
                v_bank(at_t1, st1, v1, ni, 1)
            for ni in range(NCH):
                for h in range(2):
                    if ni == NCH - 1 and h == 1:
                        continue
                    fin_bank(1, at_t1, v1, ni, h)
            # final bank as two independent 256-wide groups: the first
            # half's add+DMA drains while the second half's MMs stream
            ni, h = NCH - 1, 1
            for q in range(2):
                pq = ps_pool.tile([128, 256], F32, name="ps_f", tag="ps")
                qsl = slice(h * 512 + q * 256, h * 512 + (q + 1) * 256)
                for mi in range(NCH):
                    nc.tensor.matmul(
                        pq[:],
                        at_t1[:, mi, ni * 128 : (ni + 1) * 128],
                        v1[:, mi, qsl],
                        start=(mi == 0),
                        stop=(mi == NCH - 1),
                    )
                nc.vector.tensor_add(fins[ni][:, qsl], fins[ni][:, qsl], pq[:])
                nc.scalar.dma_start(
                    out_d[
                        ni * 128 : (ni + 1) * 128,
                        4 * h + 2 * q : 4 * h + 2 * q + 2,
                        :,
                    ],
                    fins[ni][:, qsl],
                )

    nc.compile()
    _CACHE["nc"] = nc
    return nc


def kernel(supports, inputs, state, weight, biases, output_size, _trace=False):
    supports = np.asarray(supports, dtype=np.float32)
    inputs = np.asarray(inputs, dtype=np.float32)
    state = np.asarray(state, dtype=np.float32)
    weight = np.asarray(weight, dtype=np.float32)
    biases = np.asarray(biases, dtype=np.float32)
    O_ = int(output_size)
    assert O_ == O and inputs.shape == (B, N * 64) and supports.shape == (2, N, N)

    nc = _build()

    # host staging (layout + fp16 cast): A^T, x0^T, prepped W, tiled bias row
    at_np = np.ascontiguousarray(supports.transpose(0, 2, 1)).astype(np.float16)
    x0 = np.concatenate(
        [inputs.reshape(B, N, 64), state.reshape(B, N, 64)], axis=2
    )  # [B, N, F]
    x0t = x0.transpose(0, 2, 1)  # [B, F, N] view; per-core slice made contiguous
    wk = weight.reshape(F, 5, O)
    wprep = np.stack(
        [
            wk[:, 0] - wk[:, 2] - wk[:, 4],  # What
            wk[:, 1],
            2.0 * wk[:, 2],
            wk[:, 3],
            2.0 * wk[:, 4],
        ],
        axis=1,
    )
    w16 = np.ascontiguousarray(wprep).astype(np.float16)  # [F, 5, O]
    brow = np.ascontiguousarray(np.tile(biases, BL)[None, :]).astype(np.float32)

    in_maps = []
    for c in range(NCORES):
        in_maps.append(
            {
                "x0t": np.ascontiguousarray(
                    x0t[c * BL : (c + 1) * BL]
                ).astype(np.float16),
                "at": at_np,
                "w": w16,
                "b": brow,
            }
        )

    res = run_bass_kernel_spmd(
        nc, in_maps, core_ids=list(range(NCORES)), trace=_trace
    )
    kernel.last_result = res

    # out per core: [N, BL, O] -> full [B, N*O]
    parts = [res.results[c]["out"] for c in range(NCORES)]
    full = np.concatenate(parts, axis=1)  # [N, B, O]
    return np.ascontiguousarray(full.transpose(1, 0, 2)).reshape(B, N * O_)
